# revision 12
# baseline (speedup 1.0000x reference)
"""CRF-RNN mean-field iteration kernel for Trainium2 (8 NeuronCores).

Math (per batch b, NITERS=5):
    D_norm = W / W.sum(axis=1, keepdims)          # row-normalized affinity [n, n]
    qVals  = uniqs = seg.reshape(d, n)
    loop:  Q = softmax(qVals, axis=0)             # over class dim d=21
           seg_diff   = Q @ D_norm^T              # [d, n]
           seg_update = weights @ seg_diff
           qVals      = uniqs - seg_update

Sharding: batch b -> core pair (2b, 2b+1); each core owns half the output
positions (m rows of W). The contraction runs over all n, so W^T (contraction
index on partitions) is built on-device via PE transpose-matmuls against an
identity, quantized to fp8-e4m3, and kept resident in SBUF across all 5
iterations -- W is read from HBM exactly once. The main matmuls run in fp8
DoubleRow mode (256-wide contraction per pass). Row-normalization (1/rowsum,
accumulated for free during the fp32->fp8 cast on the Scalar engine) is
applied per-partition to the tiny seg_update output.

v2 scheduling notes (from perfetto/NTFF analysis of v1):
- Small inputs (ident/wt/sel/segt) are DMA'd on the SP ring BEFORE the W
  slabs so the Scalar engine's initial-softmax ACTIVATE is not head-of-line
  blocked behind a slow ACT-ring DMA (v1 lost ~15us of startup to this).
- W streams in 32 half-slabs of [128,2048] so cast/transpose work lands
  every ~3us: the PE never sees an idle window longer than the HAM MID
  window (~3.4us), so the clock gate stays at 8/8 without junk filler
  matmuls (v1 burned ~45us of PE time on fillers and still re-throttled
  at iteration boundaries).
- The per-iteration Q exchange is a pairwise ReduceScatter with send-side
  sel-masking: core h writes qt*[h, 1-h] into the two scatter chunks, so
  RS(add) delivers exactly the partner's half to a fixed address. This
  replaces v1's AllGather + double DMA + copy_predicated select (saves
  ~3.5us/iteration of critical-path latency). The exchange is split into
  halves A (tail groups 0-1, sent mid-iteration) and B (groups 2-3) so
  collective latency hides under the next iteration's own-half matmuls.
- All collective DMAs + doorbells ride the GpSimd ring (idle otherwise)
  so they never head-of-line block the W-slab SP ring or the Scalar casts.
"""

import os
import sys

for _p in ("/opt/trn_rl_repo",):
    if _p not in sys.path:
        sys.path.insert(0, _p)

import numpy as np

BS, D, RC = 4, 21, 64
N = RC * RC       # 4096 positions
NH = N // 2       # 2048 positions per core (own half)
NT = 32           # 128-wide position tiles (global)
NTO = 16          # own position tiles
NT2 = 16          # 256-wide fp8 pair tiles (global)
HALF = 8          # pair tiles per half
HS = 32           # [128, 2048] half-slabs streamed from HBM
QPAD = 32         # class-dim padding for fp8 DoubleRow lhsT stride
NITERS = int(os.environ.get("CRF_NITERS", "5"))
NCORES = 8
RG = [[0, 1], [2, 3], [4, 5], [6, 7]]

LAST_EXEC_NS = None
_CACHE = {}


def _install_ntff_hook():
    """Best-effort registration of the axon NTFF profile hook (image antenv
    lacks axon_hooks, so trn_boot could not register it)."""
    try:
        import types

        if "antenv.axon_hooks" in sys.modules:
            return
        holder = [None]
        m = types.ModuleType("antenv.axon_hooks")
        m.set_axon_ntff_profile_hook = lambda h: holder.__setitem__(0, h)
        m.get_axon_ntff_profile_hook = lambda: holder[0]
        sys.modules["antenv.axon_hooks"] = m
        import antenv

        antenv.axon_hooks = m
        from trn_agent_boot.trn_boot import _ntff_profile_via_ctypes

        m.set_axon_ntff_profile_hook(
            _ntff_profile_via_ctypes("/opt/axon/libaxon_pjrt.so")
        )
    except Exception:
        pass


def _build(niters):
    from concourse import bacc, bass, tile, mybir

    fp32, fp16 = mybir.dt.float32, mybir.dt.float16
    sdt = mybir.dt.float8e4
    AF = mybir.ActivationFunctionType
    ALU = mybir.AluOpType
    ntile = NT2
    perf = mybir.MatmulPerfMode.DoubleRow
    XW = HALF * QPAD  # flat payload width of one exchange half

    nc = bacc.Bacc(None, target_bir_lowering=False)

    w_in = nc.dram_tensor("w", (NH, N), fp32, kind="ExternalInput")
    segt_in = nc.dram_tensor("segt", (128, NT, D), fp32, kind="ExternalInput")
    wt_in = nc.dram_tensor("wt", (D, D), fp32, kind="ExternalInput")
    sel_in = nc.dram_tensor("sel", (128, 2), fp32, kind="ExternalInput")
    id_in = nc.dram_tensor("ident", (128, 128), fp32, kind="ExternalInput")
    out_t = nc.dram_tensor("out", (128, NTO, D), fp32, kind="ExternalOutput")

    n_ex = max(0, niters - 1)
    # iteration 0 exchanges in two halves (each kicked early from within the
    # prepass); iterations >=1 use one merged exchange (the CC core
    # serializes back-to-back collectives with ~2us handoff, so two halves
    # cost ~8us more than one op)
    cc_in = []
    cc_out = []
    for k in range(n_ex):
        if k == 0:
            cc_in.append([
                nc.dram_tensor(f"cc_in{k}{hv}", (2, 128, XW), sdt, kind="Internal")
                for hv in ("a", "b")
            ])
            cc_out.append([
                nc.dram_tensor(f"cc_out{k}{hv}", (128, XW), sdt, kind="Internal")
                for hv in ("a", "b")
            ])
        else:
            cc_in.append([
                nc.dram_tensor(f"cc_in{k}", (2, 128, 2 * XW), sdt, kind="Internal")
            ])
            cc_out.append([
                nc.dram_tensor(f"cc_out{k}", (128, 2 * XW), sdt, kind="Internal")
            ])

    with tile.TileContext(nc) as tc:
        with (
            tc.tile_pool(name="wt_res", bufs=1) as wt_res,
            tc.tile_pool(name="slab32", bufs=4) as slab32p,
            tc.tile_pool(name="slab8", bufs=3) as slab8p,
            tc.tile_pool(name="state", bufs=1) as state,
            tc.tile_pool(name="qt", bufs=2) as qtp,
            tc.tile_pool(name="work", bufs=2) as work,
            tc.tile_pool(name="ps_mm", bufs=1, space=bass.MemorySpace.PSUM) as ps_mm,
            tc.tile_pool(name="ps_misc", bufs=3, space=bass.MemorySpace.PSUM) as ps_misc,
            tc.tile_pool(name="ps_junk", bufs=1, space=bass.MemorySpace.PSUM) as ps_junk,
        ):
            # ---- small inputs FIRST on the SP ring (slabs follow) --------
            id32 = state.tile([128, 128], fp32)
            nc.sync.dma_start(id32[:], id_in[:])
            wt32 = state.tile([D, D], fp32)
            nc.sync.dma_start(wt32[:], wt_in[:])
            selt = state.tile([128, 2], fp32)
            nc.sync.dma_start(selt[:], sel_in[:])
            segt = state.tile([128, NT, D], fp32)
            nc.sync.dma_start(segt[:], segt_in[:])

            id_s = state.tile([128, 128], sdt)
            nc.vector.tensor_copy(id_s[:], id32[:])
            wt16 = state.tile([D, D], fp16)
            nc.gpsimd.tensor_copy(wt16[:], wt32[:])
            sel8 = state.tile([128, 2], sdt)
            nc.gpsimd.tensor_copy(sel8[:], selt[:])
            zbias = state.tile([128, 1], fp32)
            nc.gpsimd.memset(zbias[:], 0.0)

            # ---- initial Q = softmax(uniqs) over all 32 tiles ------------
            ex0 = state.tile([128, NT, D], fp32)
            nc.scalar.activation(ex0[:], segt[:], AF.Exp, bias=zbias[:])
            ssum0 = state.tile([128, NT], fp32)
            nc.vector.reduce_sum(ssum0[:], ex0[:], axis=mybir.AxisListType.X)
            srecip0 = state.tile([128, NT], fp32)
            nc.vector.reciprocal(srecip0[:], ssum0[:])
            qt_own = qtp.tile([128, NTO, QPAD], sdt, tag="qt_own", name="qt_own0")
            qt_par0 = qtp.tile([128, NTO, QPAD], sdt, tag="qt_par0", name="qt_par0")
            nc.vector.tensor_tensor(
                qt_own[:, :, 0:D],
                ex0[:, 0:NTO, :],
                srecip0[:, 0:NTO, None].broadcast_to((128, NTO, D)),
                ALU.mult,
            )
            nc.vector.tensor_tensor(
                qt_par0[:, :, 0:D],
                ex0[:, NTO:NT, :],
                srecip0[:, NTO:NT, None].broadcast_to((128, NTO, D)),
                ALU.mult,
            )

            # ---- resident W^T (fp8, pair-interleaved for DoubleRow) ------
            # wt_mc[mc][p, t2, i, j] = W^T[256*t2 + 128*i + p, 512*mc + j]
            wt_mc = [
                wt_res.tile([128, NT2, 2, 512], sdt, tag=f"wtr{mc}", name=f"wt_mc{mc}")
                for mc in range(4)
            ]
            rs8 = [
                state.tile([128, 4, 2], fp32, tag=f"rs8_{g}", name=f"rs8_{g}")
                for g in range(4)
            ]
            rs_sum = [
                state.tile([128, 4], fp32, tag=f"rssum{g}", name=f"rs_sum{g}")
                for g in range(4)
            ]
            rs_rec = [
                state.tile([128, 4], fp32, tag=f"rsrec{g}", name=f"rs_rec{g}")
                for g in range(4)
            ]

            def lhs_of(t, q_own, q_pa, q_pb):
                if t < HALF:
                    src, j = q_own, t
                elif t < HALF + 4:
                    src, j = q_pa, t - HALF
                else:
                    src, j = q_pb, t - HALF - 4
                return src[:, 2 * j : 2 * j + 2, 0:D]

            class IterEmitter:
                """Emits one mean-field iteration in dependency-friendly
                pieces so matmuls, evacuations, and the softmax tail
                pipeline across engines (and, for iteration 0, interleave
                with the prepass)."""

                def __init__(self, it, q_own, q_pa, q_pb, last):
                    self.it, self.last = it, last
                    self.q_own, self.q_pa, self.q_pb = q_own, q_pa, q_pb
                    # one PSUM tile per 512-col output group: keeps the
                    # accumulate/evacuate dependency per-group so group
                    # mc+1's matmuls never serialize behind group mc's
                    # evacuation (whole-tile WAR observed in v2 trace)
                    self.pP = [
                        ps_mm.tile([D, 512], fp32, tag=f"pp{mc}", name=f"pp{it}_{mc}")
                        for mc in range(4)
                    ]
                    self.ps16g = {}
                    self.pUTg = {}
                    self.qt_next = None
                    if not last:
                        self.qt_next = qtp.tile(
                            [128, NTO, QPAD], sdt, tag="qt_own", name=f"qt_own{it+1}"
                        )

                def phase(self, mms):
                    for t, mc in mms:
                        nc.tensor.matmul(
                            self.pP[mc][:],
                            lhs_of(t, self.q_own, self.q_pa, self.q_pb),
                            wt_mc[mc][:, t, :, :],
                            start=(t == 0),
                            stop=(t == ntile - 1),
                            perf_mode=perf,
                        )

                def evac(self, mc):
                    t16 = work.tile(
                        [D, 512], fp16, tag=f"ps16_{mc}", name=f"ps16_{self.it}_{mc}"
                    )
                    nc.vector.tensor_copy(t16[:], self.pP[mc][:])
                    self.ps16g[mc] = t16

                def ut(self, g):
                    pu = ps_misc.tile(
                        [128, 4 * D], fp32, tag="misc", name=f"pUT{self.it}_{g}"
                    )
                    for jj in range(4):
                        nc.tensor.matmul(
                            pu[:, jj * D : (jj + 1) * D],
                            self.ps16g[g][:, jj * 128 : (jj + 1) * 128],
                            wt16[:],
                            start=True,
                            stop=True,
                        )
                    self.pUTg[g] = pu

                def tail(self, g):
                    it, sl = self.it, slice(4 * g, 4 * g + 4)
                    upd = work.tile([128, 4, D], fp32, tag=f"upd{g}", name=f"upd{it}_{g}")
                    nc.vector.tensor_tensor(
                        upd[:],
                        self.pUTg[g][:].rearrange("p (a b) -> p a b", a=4),
                        rs_rec[g][:, :, None].broadcast_to((128, 4, D)),
                        ALU.mult,
                    )
                    qv = work.tile([128, 4, D], fp32, tag=f"qv{g}", name=f"qv{it}_{g}")
                    nc.vector.tensor_tensor(qv[:], segt[:, sl, :], upd[:], ALU.subtract)
                    if self.last:
                        nc.sync.dma_start(out_t[:, sl, :], qv[:])
                        return
                    exq = work.tile([128, 4, D], fp32, tag=f"exq{g}", name=f"exq{it}_{g}")
                    nc.scalar.activation(exq[:], qv[:], AF.Exp, bias=zbias[:])
                    ssum = work.tile([128, 4], fp32, tag=f"ssum{g}", name=f"ssum{it}_{g}")
                    nc.vector.reduce_sum(ssum[:], exq[:], axis=mybir.AxisListType.X)
                    srec = work.tile([128, 4], fp32, tag=f"srec{g}", name=f"srec{it}_{g}")
                    nc.vector.reciprocal(srec[:], ssum[:])
                    nc.vector.tensor_tensor(
                        self.qt_next[:, sl, 0:D],
                        exq[:],
                        srec[:, :, None].broadcast_to((128, 4, D)),
                        ALU.mult,
                    )

            def send(k, hv, qt_next, eng):
                """Mask qt half hv by sel and kick the pairwise ReduceScatter.
                Core h contributes its data only to the partner's scatter
                chunk, so RS(add) yields exactly the partner's half. `eng` is
                the masking engine: gpsimd for iteration 0 (the Vector queue
                is congested with prepass transpose copies), vector later."""
                if hv is None:
                    sl, wid = slice(0, NTO), 2 * XW
                    ci, co = cc_in[k][0], cc_out[k][0]
                else:
                    sl = slice(0, HALF) if hv == 0 else slice(HALF, NTO)
                    wid = XW
                    ci, co = cc_in[k][hv], cc_out[k][hv]
                msk = work.tile(
                    [128, 2, wid], sdt, tag=f"msk{hv}", name=f"msk{k}_{hv}"
                )
                qflat = qt_next[:, sl, :].rearrange("p a b -> p (a b)")
                for s in range(2):
                    eng.tensor_tensor(
                        msk[:, s, :],
                        qflat,
                        sel8[:, s : s + 1].broadcast_to((128, wid)),
                        ALU.mult,
                    )
                nc.gpsimd.dma_start(ci[:].rearrange("s p x -> p s x"), msk[:])
                nc.gpsimd.collective_compute(
                    "ReduceScatter",
                    ALU.add,
                    replica_groups=RG,
                    ins=[ci[:].opt()],
                    outs=[co[:].opt()],
                )

            def recv_pair(k, it):
                """Receive partner tiles for iteration `it` from exchange k
                into two SBUF tiles (A: partner pair-tiles 0-3, B: 4-7)."""
                qa = qtp.tile([128, HALF, QPAD], sdt, tag="qpA", name=f"qpA_{it}")
                qb = qtp.tile([128, HALF, QPAD], sdt, tag="qpB", name=f"qpB_{it}")
                if k == 0:
                    nc.gpsimd.dma_start(
                        qa[:].rearrange("p a b -> p (a b)"), cc_out[k][0][:]
                    )
                    nc.gpsimd.dma_start(
                        qb[:].rearrange("p a b -> p (a b)"), cc_out[k][1][:]
                    )
                else:
                    nc.gpsimd.dma_start(
                        qa[:].rearrange("p a b -> p (a b)"), cc_out[k][0][:, 0:XW]
                    )
                    nc.gpsimd.dma_start(
                        qb[:].rearrange("p a b -> p (a b)"), cc_out[k][0][:, XW : 2 * XW]
                    )
                return qa, qb

            # ---- prepass (half-slabs, transpose, rowsum) + iteration 0 ---
            em = IterEmitter(
                0, qt_own, qt_par0[:, 0:HALF, :], qt_par0[:, HALF:NTO, :],
                last=(niters == 1),
            )
            for hs in range(HS):
                ms, hh = hs // 2, hs % 2
                w32 = slab32p.tile([128, NH], fp32, tag="w32", name=f"w32_{hs}")
                nc.sync.dma_start(
                    w32[:], w_in[ms * 128 : (ms + 1) * 128, hh * NH : (hh + 1) * NH]
                )
                w8 = slab8p.tile([128, NH], sdt, tag="w8", name=f"w8_{hs}")
                nc.scalar.activation(
                    w8[:], w32[:], AF.Copy,
                    accum_out=rs8[ms // 4][:, ms % 4, hh : hh + 1],
                )
                mc, col = ms // 4, (ms % 4) * 128
                for g in range(4):
                    ptp = ps_misc.tile([128, 512], fp32, tag="misc", name=f"ptp{hs}_{g}")
                    for k2 in range(4):
                        nt = 4 * g + k2
                        nc.tensor.matmul(
                            ptp[:, k2 * 128 : (k2 + 1) * 128],
                            w8[:, nt * 128 : (nt + 1) * 128],
                            id_s[:],
                            start=True,
                            stop=True,
                        )
                    dst = wt_mc[mc][:, 8 * hh + 2 * g : 8 * hh + 2 * g + 2, :, col : col + 128]
                    src = ptp[:].rearrange("p (a b c) -> p a b c", a=2, b=2)
                    if g == 3:
                        # offload 1/4 of PSUM->SBUF evacuations to Scalar so
                        # the Vector queue drains near-realtime (v3: a ~25us
                        # Vector backlog delayed iter0 tails and the first
                        # exchange by that much)
                        nc.scalar.activation(dst, src, AF.Copy)
                    else:
                        nc.vector.tensor_copy(dst, src)
                if hs % 8 == 7:
                    g = hs // 8
                    nc.vector.reduce_sum(
                        rs_sum[g][:], rs8[g][:], axis=mybir.AxisListType.X
                    )
                    nc.vector.reciprocal(rs_rec[g][:], rs_sum[g][:])
                    em.phase([(t, g) for t in range(ntile)])
                    em.evac(g)
                    if g >= 1:
                        em.ut(g - 1)
                        em.tail(g - 1)
                        if g == 2 and n_ex > 0:
                            send(0, 0, em.qt_next, nc.gpsimd)
            em.ut(3)
            em.tail(3)
            if n_ex > 0:
                send(0, 1, em.qt_next, nc.gpsimd)
                qpA, qpB = recv_pair(0, 1)
                qt_own = em.qt_next

            # keep-warm junk matmuls: anchor a little PE activity into the
            # windows where the tensor queue head waits on an exchange, so
            # the HAM duty-cycle monitor doesn't drop the clock to 4/8
            junk = ps_junk.tile([D, 512], fp32, name="junk")

            def fillers(n):
                for f in range(n):
                    nc.tensor.matmul(
                        junk[:], id_s[:, 0:D], wt_mc[f % 4][:, f % NT2, 0, :],
                        start=True, stop=True,
                    )

            # ---- iterations 1..niters-1 ---------------------------------
            for it in range(1, niters):
                em = IterEmitter(it, qt_own, qpA, qpB, last=(it == niters - 1))
                em.phase([(t, mc) for t in range(HALF) for mc in range(4)])
                fillers(2)
                em.phase([(t, mc) for t in range(HALF, HALF + 4) for mc in range(4)])
                fillers(4)
                for mc in range(4):
                    em.phase([(t, mc) for t in range(HALF + 4, ntile)])
                    em.evac(mc)
                    if mc >= 1:
                        em.ut(mc - 1)
                        em.tail(mc - 1)
                em.ut(3)
                em.tail(3)
                if it < niters - 1:
                    send(it, None, em.qt_next, nc.vector)
                    qpA, qpB = recv_pair(it, it + 1)
                    qt_own = em.qt_next

    nc.compile()
    return nc


def _get_nc(niters):
    if niters not in _CACHE:
        _CACHE[niters] = _build(niters)
    return _CACHE[niters]


def kernel(seg, W, weights):
    global LAST_EXEC_NS
    assert seg.shape == (BS, D, RC, RC) and W.shape == (BS, N, N)
    trace = bool(os.environ.get("BASS_TRACE"))
    if trace:
        _install_ntff_hook()

    from concourse.bass_utils import run_bass_kernel_spmd

    nc = _get_nc(NITERS)

    seg32 = np.ascontiguousarray(seg, dtype=np.float32)
    W32 = np.ascontiguousarray(W, dtype=np.float32)
    wt_np = np.ascontiguousarray(weights.T, dtype=np.float32)
    id_np = np.eye(128, dtype=np.float32)

    in_maps = []
    for c in range(NCORES):
        b, h = c // 2, c % 2
        own = slice(NH * h, NH * h + NH)
        par = slice(NH * (1 - h), NH * (1 - h) + NH)
        Wb = W32[b]
        w_np = np.ascontiguousarray(
            np.concatenate([Wb[own, own], Wb[own, par]], axis=1)
        )
        st = seg32[b].reshape(D, N).T  # [n, d]
        st_perm = np.concatenate([st[own], st[par]], axis=0)
        segt_np = np.ascontiguousarray(
            st_perm.reshape(NT, 128, D).transpose(1, 0, 2)
        )
        # ReduceScatter chunk masks: core h sends its qt into chunk s
        # iff s == 1-h, i.e. mask = [h, 1-h].
        sel_np = np.zeros((128, 2), np.float32)
        sel_np[:, 0] = float(h)
        sel_np[:, 1] = float(1 - h)
        in_maps.append(
            {"w": w_np, "segt": segt_np, "wt": wt_np, "sel": sel_np, "ident": id_np}
        )

    res = run_bass_kernel_spmd(
        nc, in_maps, core_ids=list(range(NCORES)), trace=trace
    )
    LAST_EXEC_NS = res.exec_time_ns

    out = np.empty((BS, D, N), np.float32)
    for c in range(NCORES):
        b, h = c // 2, c % 2
        qv = res.results[c]["out"]  # [128, NTO, D]
        block = qv.transpose(2, 1, 0).reshape(D, NH)
        out[b][:, NH * h : NH * h + NH] = block
    return out.reshape(BS, D, RC, RC)


if __name__ == "__main__":
    rng = np.random.default_rng(0)
    seg = rng.standard_normal((BS, D, RC, RC)).astype(np.float32)
    W = rng.random((BS, N, N), dtype=np.float32)
    weights = rng.standard_normal((D, D)).astype(np.float32)
    out = kernel(seg=seg, W=W, weights=weights)
    print("out", out.shape, out.dtype, float(np.abs(out).mean()))
